# revision 3
# baseline (speedup 1.0000x reference)
"""Trainium2 Bass kernel for nn_DecSwitchedFC (MoE hard routing).

Math (per token b, expert e = y_idx[b]):
    out[b] = x[b] + z[b, e] * (relu(x[b] @ W1[e] + b1[e]) @ W2[e] + b2[e])

Strategy: expert-parallel over 8 NeuronCores, 2 experts per core.  The host
routes tokens to their experts (pure data movement — gather/scatter and
transpose), each core runs the two dense bottleneck FCs for its experts'
tokens in fp32 on the tensor engine, applies bias/relu/route-scale/residual
on the scalar/vector engines, and the host scatters rows back.  Only the
selected expert is computed per token (1/16 of the reference FLOPs).

Device data layout is fully "feature-major" (d or h on partitions, tokens on
the free axis) so no on-device transposes are needed:
    h^T[256, n]  = W1[e]^T(lhsT=W1) @ x^T          (K=1024, 8 chunks)
    o^T[1024, n] = W2[e]^T(lhsT=W2) @ relu(h^T+b1) (K=256, 2 chunks)
    out^T        = (o^T + b2) * z + x^T
"""

import numpy as np

import concourse.bacc as bacc
import concourse.mybir as mybir
import concourse.tile as tile
from concourse.bass_utils import run_bass_kernel_spmd

D = 1024        # model dim
H = 256         # bottleneck dim
NB = 16         # n experts
NCORES = 8
EPC = NB // NCORES   # experts per core
TILE_N = 512    # token-tile width (fp32 moving-operand max)
KC1 = D // 128  # contraction chunks for x @ W1
KC2 = H // 128  # contraction chunks for h @ W2
F32 = mybir.dt.float32

_build_cache: dict[int, object] = {}
LAST_RESULTS = None  # BassKernelResults of the most recent run (for profiling)


def _build(cap: int):
    """One SPMD program; `cap` = padded token capacity per expert."""
    if cap in _build_cache:
        return _build_cache[cap]
    C = EPC * cap
    nc = bacc.Bacc("TRN2", target_bir_lowering=False, debug=False)

    xg = nc.dram_tensor("xg", [D, C], F32, kind="ExternalInput")
    zg = nc.dram_tensor("zg", [128, C], F32, kind="ExternalInput")
    w1 = nc.dram_tensor("w1", [EPC, D, H], F32, kind="ExternalInput")
    w2 = nc.dram_tensor("w2", [EPC, H, D], F32, kind="ExternalInput")
    # bias[p, e*10 + j]     = b1[e, 128j + p]   (j in 0..1)
    # bias[p, e*10 + 2 + i] = b2[e, 128i + p]   (i in 0..7)
    bias = nc.dram_tensor("bias", [128, EPC * (KC2 + KC1)], F32,
                          kind="ExternalInput")
    outT = nc.dram_tensor("outT", [D, C], F32, kind="ExternalOutput")

    xg_r = xg.rearrange("(k p) c -> p k c", p=128)
    w1_r = w1.rearrange("e (k p) m -> p e k m", p=128)
    w2_r = w2.rearrange("e (k p) m -> p e k m", p=128)

    with tile.TileContext(nc) as tc:
        with (
            tc.tile_pool(name="const", bufs=1) as cpool,
            tc.tile_pool(name="w1p", bufs=2) as w1pool,
            tc.tile_pool(name="w2p", bufs=2) as w2pool,
            tc.tile_pool(name="xp", bufs=3) as xpool,
            tc.tile_pool(name="zp", bufs=3) as zpool,
            tc.tile_pool(name="hp", bufs=2) as hpool,
            tc.tile_pool(name="op", bufs=4) as opool,
            tc.tile_pool(name="ph", bufs=2, space="PSUM") as phpool,
            tc.tile_pool(name="po", bufs=3, space="PSUM") as popool,
        ):
            bias_t = cpool.tile([128, EPC * (KC2 + KC1)], F32)
            nc.sync.dma_start(bias_t[:], bias[:])

            for s in range(EPC):
                w1t = w1pool.tile([128, KC1, H], F32, tag="w1t")
                nc.sync.dma_start(w1t[:], w1_r[:, s])
                w2t = w2pool.tile([128, KC2, D], F32, tag="w2t")
                nc.sync.dma_start(w2t[:], w2_r[:, s])

                for t0 in range(0, cap, TILE_N):
                    tn = min(TILE_N, cap - t0)
                    c0 = s * cap + t0

                    xt = xpool.tile([128, KC1, tn], F32, tag="xt")
                    nc.sync.dma_start(xt[:], xg_r[:, :, c0:c0 + tn])
                    zt = zpool.tile([128, tn], F32, tag="zt")
                    nc.sync.dma_start(zt[:], zg[:, c0:c0 + tn])

                    ht = hpool.tile([128, KC2, tn], F32, tag="ht")
                    for j in range(KC2):
                        ph = phpool.tile([128, tn], F32, tag="ph")
                        for k in range(KC1):
                            nc.tensor.matmul(
                                ph[:], w1t[:, k, 128 * j:128 * (j + 1)],
                                xt[:, k, :],
                                start=(k == 0), stop=(k == KC1 - 1))
                        nc.scalar.activation(
                            ht[:, j, :], ph[:],
                            mybir.ActivationFunctionType.Relu,
                            bias=bias_t[:, s * 10 + j:s * 10 + j + 1])

                    for i in range(KC1):
                        po = popool.tile([128, tn], F32, tag="po")
                        for j in range(KC2):
                            nc.tensor.matmul(
                                po[:], w2t[:, j, 128 * i:128 * (i + 1)],
                                ht[:, j, :],
                                start=(j == 0), stop=(j == KC2 - 1))
                        ot = opool.tile([128, tn], F32, tag="ot")
                        # (o + b2) * z
                        nc.vector.scalar_tensor_tensor(
                            ot[:], po[:],
                            bias_t[:, s * 10 + 2 + i:s * 10 + 3 + i],
                            zt[:],
                            mybir.AluOpType.add, mybir.AluOpType.mult)
                        # + x (residual)
                        nc.vector.tensor_add(ot[:], ot[:], xt[:, i, :])
                        nc.sync.dma_start(
                            outT[128 * i:128 * (i + 1), c0:c0 + tn], ot[:])

    nc.compile()
    _build_cache[cap] = nc
    return nc


def kernel(x, y_idx, y, z, W1, b1, W2, b2):
    x = np.ascontiguousarray(np.asarray(x, dtype=np.float32))
    z = np.asarray(z, dtype=np.float32)
    W1 = np.asarray(W1, dtype=np.float32)
    b1 = np.asarray(b1, dtype=np.float32)
    W2 = np.asarray(W2, dtype=np.float32)
    b2 = np.asarray(b2, dtype=np.float32)
    e = np.asarray(y_idx).reshape(-1).astype(np.int64)
    B = x.shape[0]

    idxs = [np.flatnonzero(e == k) for k in range(NB)]
    cap = max(128, -(-max(len(i) for i in idxs) // 128) * 128)
    C = EPC * cap

    nc = _build(cap)

    nbias = EPC * (KC2 + KC1)
    in_maps = []
    for c in range(NCORES):
        xg = np.zeros((D, C), np.float32)
        zg = np.zeros((128, C), np.float32)
        bias = np.zeros((128, nbias), np.float32)
        for s in range(EPC):
            k = EPC * c + s
            idx = idxs[k]
            n = len(idx)
            xg[:, s * cap:s * cap + n] = x[idx].T
            zg[:, s * cap:s * cap + n] = z[idx, k][None, :]
            bias[:, s * 10:s * 10 + KC2] = b1[k].reshape(KC2, 128).T
            bias[:, s * 10 + KC2:s * 10 + KC2 + KC1] = b2[k].reshape(KC1, 128).T
        in_maps.append({
            "xg": xg,
            "zg": zg,
            "w1": np.ascontiguousarray(W1[EPC * c:EPC * (c + 1)]),
            "w2": np.ascontiguousarray(W2[EPC * c:EPC * (c + 1)]),
            "bias": bias,
        })

    res = run_bass_kernel_spmd(nc, in_maps, core_ids=list(range(NCORES)))
    global LAST_RESULTS
    LAST_RESULTS = res

    out = np.empty((B, D), np.float32)
    for c in range(NCORES):
        outT = res.results[c]["outT"]
        for s in range(EPC):
            k = EPC * c + s
            idx = idxs[k]
            out[idx] = outT[:, s * cap:s * cap + len(idx)].T
    return out


# revision 6
# speedup vs baseline: 1.0036x; 1.0036x over previous
"""Trainium2 Bass kernel for nn_DecSwitchedFC (MoE hard routing).

Math (per token b, expert e = y_idx[b]):
    out[b] = x[b] + z[b, e] * (relu(x[b] @ W1[e] + b1[e]) @ W2[e] + b2[e])

Strategy: expert-parallel over 8 NeuronCores, 2 experts per core.  The host
routes tokens to their experts (pure data movement — gather/scatter and
transpose), each core runs the two dense bottleneck FCs for its experts'
tokens on the tensor engine, applies bias/relu/route-scale/residual on the
scalar/vector engines, and the host scatters rows back.  Only the selected
expert is computed per token (1/16 of the reference FLOPs).

Device data layout is fully "feature-major" (d or h on partitions, tokens on
the free axis) so no on-device transposes are needed:
    h^T[256, n]  = W1[e]^T(lhsT=W1) @ x^T          (K=1024, 8 chunks)
    o^T[1024, n] = W2[e]^T(lhsT=W2) @ relu(h^T+b1) (K=256, 2 chunks)
    out^T        = (o^T + b2) * z + x^T

Host-side arrays are packed so every DMA is 128 descriptors of >=1KB
contiguous bytes (one per SBUF partition).
"""

import numpy as np

import concourse.bacc as bacc
import concourse.mybir as mybir
import concourse.tile as tile
from concourse.bass_utils import run_bass_kernel_spmd

D = 1024        # model dim
H = 256         # bottleneck dim
NB = 16         # n experts
NCORES = 8
EPC = NB // NCORES   # experts per core
TILE_N = 512    # token-tile width (fp32 moving-operand max)
KC1 = D // 128  # contraction chunks for x @ W1
KC2 = H // 128  # contraction chunks for h @ W2
F32 = mybir.dt.float32
# Matmul operand dtype: float32 (exact, 4 cyc/row) or float32r
# (TF32-like ~1e-4 relative error, 1 cyc/row).
MM_DT = F32

_build_cache: dict[tuple, object] = {}
LAST_RESULTS = None  # BassKernelResults of the most recent run (for profiling)


def _tile_seq(cap: int):
    """Global tile order: small tile first (fast PE start) and small tile
    last (short drain tail).  Returns [(s, t0, tn), ...]."""
    tiles = [(t0, min(TILE_N, cap - t0)) for t0 in range(0, cap, TILE_N)]
    first = sorted(tiles, key=lambda t: t[1])          # ascending width
    last = sorted(tiles, key=lambda t: -t[1])          # descending width
    seq = [(0, t0, tn) for t0, tn in first]
    seq += [(1, t0, tn) for t0, tn in last]
    return seq


def _build(cap: int):
    key = (cap, MM_DT)
    if key in _build_cache:
        return _build_cache[key]
    C = EPC * cap
    seq = _tile_seq(cap)
    xcols = KC1 * cap * EPC   # total packed x columns

    nc = bacc.Bacc("TRN2", target_bir_lowering=False, debug=False)

    xg = nc.dram_tensor("xg", [128, xcols], F32, kind="ExternalInput")
    zg = nc.dram_tensor("zg", [128, C], F32, kind="ExternalInput")
    w1 = nc.dram_tensor("w1", [128, EPC, KC1 * H], F32, kind="ExternalInput")
    w2 = nc.dram_tensor("w2", [128, EPC, KC2 * D], F32, kind="ExternalInput")
    # bias[p, e*10 + j]     = b1[e, 128j + p]   (j in 0..1)
    # bias[p, e*10 + 2 + i] = b2[e, 128i + p]   (i in 0..7)
    bias = nc.dram_tensor("bias", [128, EPC * (KC2 + KC1)], F32,
                          kind="ExternalInput")
    outT = nc.dram_tensor("outT", [D, C], F32, kind="ExternalOutput")
    outT_r = outT.rearrange("(i p) c -> p i c", p=128)

    with tile.TileContext(nc) as tc:
        with (
            tc.tile_pool(name="const", bufs=1) as cpool,
            tc.tile_pool(name="w1p", bufs=2) as w1pool,
            tc.tile_pool(name="w2p", bufs=2) as w2pool,
            tc.tile_pool(name="xp", bufs=3) as xpool,
            tc.tile_pool(name="zp", bufs=3) as zpool,
            tc.tile_pool(name="hp", bufs=2) as hpool,
            tc.tile_pool(name="op", bufs=6) as opool,
            tc.tile_pool(name="ph", bufs=2, space="PSUM") as phpool,
            tc.tile_pool(name="po", bufs=3, space="PSUM") as popool,
        ):
            bias_t = None
            w1t = w2t = None
            cur_s = -1
            xoff = 0
            for q, (s, t0, tn) in enumerate(seq):
                if s != cur_s:
                    cur_s = s
                    w1t = w1pool.tile([128, KC1, H], MM_DT, tag="w1t")
                    for k in range(KC1):
                        nc.sync.dma_start(w1t[:, k, :],
                                          w1[:, s, k * H:(k + 1) * H])
                    w2t = w2pool.tile([128, KC2, D], MM_DT, tag="w2t")
                    for j in range(KC2):
                        nc.sync.dma_start(w2t[:, j, :],
                                          w2[:, s, j * D:(j + 1) * D])

                xt = xpool.tile([128, KC1, tn], MM_DT, tag="xt")
                nc.sync.dma_start(
                    xt[:],
                    xg[:, xoff:xoff + KC1 * tn].rearrange(
                        "p (k c) -> p k c", k=KC1))
                c0 = s * cap + t0
                zt = zpool.tile([128, tn], F32, tag="zt")
                nc.sync.dma_start(zt[:], zg[:, c0:c0 + tn])
                if bias_t is None:
                    bias_t = cpool.tile([128, EPC * (KC2 + KC1)], F32)
                    nc.sync.dma_start(bias_t[:], bias[:])

                ht = hpool.tile([128, KC2, tn], MM_DT, tag="ht")
                for j in range(KC2):
                    ph = phpool.tile([128, tn], F32, tag="ph")
                    for k in range(KC1):
                        nc.tensor.matmul(
                            ph[:], w1t[:, k, 128 * j:128 * (j + 1)],
                            xt[:, k, :],
                            start=(k == 0), stop=(k == KC1 - 1))
                    nc.scalar.activation(
                        ht[:, j, :], ph[:],
                        mybir.ActivationFunctionType.Relu,
                        bias=bias_t[:, s * 10 + j:s * 10 + j + 1])

                for i in range(KC1):
                    po = popool.tile([128, tn], F32, tag="po")
                    for j in range(KC2):
                        nc.tensor.matmul(
                            po[:], w2t[:, j, 128 * i:128 * (i + 1)],
                            ht[:, j, :],
                            start=(j == 0), stop=(j == KC2 - 1))
                    ot = opool.tile([128, tn], F32, tag="ot")
                    # (o + b2) * z
                    nc.vector.scalar_tensor_tensor(
                        ot[:], po[:],
                        bias_t[:, s * 10 + 2 + i:s * 10 + 3 + i],
                        zt[:],
                        mybir.AluOpType.add, mybir.AluOpType.mult)
                    # + x (residual)
                    nc.vector.tensor_add(ot[:], ot[:], xt[:, i, :])
                    nc.gpsimd.dma_start(outT_r[:, i, c0:c0 + tn], ot[:])

                xoff += KC1 * tn

    nc.compile()
    _build_cache[key] = nc
    return nc


def kernel(x, y_idx, y, z, W1, b1, W2, b2):
    x = np.ascontiguousarray(np.asarray(x, dtype=np.float32))
    z = np.asarray(z, dtype=np.float32)
    W1 = np.asarray(W1, dtype=np.float32)
    b1 = np.asarray(b1, dtype=np.float32)
    W2 = np.asarray(W2, dtype=np.float32)
    b2 = np.asarray(b2, dtype=np.float32)
    e = np.asarray(y_idx).reshape(-1).astype(np.int64)
    B = x.shape[0]

    idxs = [np.flatnonzero(e == k) for k in range(NB)]
    cap = max(128, -(-max(len(i) for i in idxs) // 128) * 128)
    C = EPC * cap
    seq = _tile_seq(cap)
    xcols = KC1 * cap * EPC

    nc = _build(cap)

    nbias = EPC * (KC2 + KC1)
    in_maps = []
    for c in range(NCORES):
        xg = np.zeros((128, xcols), np.float32)
        zg = np.zeros((128, C), np.float32)
        bias = np.zeros((128, nbias), np.float32)
        w1 = np.empty((128, EPC, KC1 * H), np.float32)
        w2 = np.empty((128, EPC, KC2 * D), np.float32)
        for s in range(EPC):
            k = EPC * c + s
            idx = idxs[k]
            n = len(idx)
            # z replicated over partitions, token-major at s*cap
            zg[:, s * cap:s * cap + n] = z[idx, k][None, :]
            bias[:, s * 10:s * 10 + KC2] = b1[k].reshape(KC2, 128).T
            bias[:, s * 10 + KC2:s * 10 + KC2 + KC1] = b2[k].reshape(KC1, 128).T
            # weights packed [p, k*H + m] / [p, j*D + m]
            w1[:, s] = W1[k].reshape(KC1, 128, H).transpose(1, 0, 2).reshape(
                128, KC1 * H)
            w2[:, s] = W2[k].reshape(KC2, 128, D).transpose(1, 0, 2).reshape(
                128, KC2 * D)
        # x packed per tile block: [p, k*tn + col]
        xoff = 0
        for s, t0, tn in seq:
            k = EPC * c + s
            idx = idxs[k]
            seg = idx[t0:t0 + tn]
            n = len(seg)
            if n:
                full = np.zeros((128, KC1, tn), np.float32)
                full[:, :, :n] = x[seg].reshape(n, KC1, 128).transpose(2, 1, 0)
                xg[:, xoff:xoff + KC1 * tn] = full.reshape(128, KC1 * tn)
            xoff += KC1 * tn
        in_maps.append({"xg": xg, "zg": zg, "w1": w1, "w2": w2, "bias": bias})

    res = run_bass_kernel_spmd(nc, in_maps, core_ids=list(range(NCORES)))
    global LAST_RESULTS
    LAST_RESULTS = res

    out = np.empty((B, D), np.float32)
    for c in range(NCORES):
        outT = res.results[c]["outT"]
        for s in range(EPC):
            k = EPC * c + s
            idx = idxs[k]
            out[idx] = outT[:, s * cap:s * cap + len(idx)].T
    return out


# revision 7
# speedup vs baseline: 1.1006x; 1.0966x over previous
"""Trainium2 Bass kernel for nn_DecSwitchedFC (MoE hard routing).

Math (per token b, expert e = y_idx[b]):
    out[b] = x[b] + z[b, e] * (relu(x[b] @ W1[e] + b1[e]) @ W2[e] + b2[e])

Strategy: expert-parallel over 8 NeuronCores, 2 experts per core.  The host
routes tokens to their experts (pure data movement — gather/scatter and
transpose), each core runs the two dense bottleneck FCs for its experts'
tokens on the tensor engine, applies bias/relu/route-scale/residual on the
scalar/vector engines, and the host scatters rows back.  Only the selected
expert is computed per token (1/16 of the reference FLOPs).

Device data layout is fully "feature-major" (d or h on partitions, tokens on
the free axis) so no on-device transposes are needed:
    h^T[256, n]  = W1[e]^T(lhsT=W1) @ x^T          (K=1024, 8 chunks)
    o^T[1024, n] = W2[e]^T(lhsT=W2) @ relu(h^T+b1) (K=256, 2 chunks)
    out^T        = (o^T + b2) * z + x^T

All host-side arrays are packed so every DMA is a single issue with 128
descriptors of contiguous bytes (one per SBUF partition) — DMA issue rate,
not bandwidth, limits the pipeline ramp otherwise.
"""

import numpy as np

import concourse.bacc as bacc
import concourse.mybir as mybir
import concourse.tile as tile
from concourse.bass_utils import run_bass_kernel_spmd

D = 1024        # model dim
H = 256         # bottleneck dim
NB = 16         # n experts
NCORES = 8
EPC = NB // NCORES   # experts per core
TILE_N = 512    # token-tile width (fp32 moving-operand max)
KC1 = D // 128  # contraction chunks for x @ W1
KC2 = H // 128  # contraction chunks for h @ W2
F32 = mybir.dt.float32
# Matmul operand dtype: float32 (exact, 4 cyc/row) or float32r
# (TF32-like ~1e-4 relative error, 1 cyc/row).
MM_DT = F32

_build_cache: dict[tuple, object] = {}
LAST_RESULTS = None  # BassKernelResults of the most recent run (for profiling)


def _tile_seq(cap: int):
    """Global tile order: small tile first (fast PE start) and small tile
    last (short drain tail).  Returns [(s, t0, tn), ...]."""
    tiles = [(t0, min(TILE_N, cap - t0)) for t0 in range(0, cap, TILE_N)]
    first = sorted(tiles, key=lambda t: t[1])          # ascending width
    last = sorted(tiles, key=lambda t: -t[1])          # descending width
    seq = [(0, t0, tn) for t0, tn in first]
    seq += [(1, t0, tn) for t0, tn in last]
    return seq


def _build(cap: int):
    key = (cap, MM_DT)
    if key in _build_cache:
        return _build_cache[key]
    C = EPC * cap
    seq = _tile_seq(cap)
    xcols = KC1 * cap * EPC   # total packed x / out columns

    nc = bacc.Bacc("TRN2", target_bir_lowering=False, debug=False)

    xg = nc.dram_tensor("xg", [128, xcols], MM_DT, kind="ExternalInput")
    zg = nc.dram_tensor("zg", [128, C], F32, kind="ExternalInput")
    w1 = nc.dram_tensor("w1", [128, EPC, KC1 * H], MM_DT, kind="ExternalInput")
    w2 = nc.dram_tensor("w2", [128, EPC, KC2 * D], MM_DT, kind="ExternalInput")
    # bias[p, e*10 + j]     = b1[e, 128j + p]   (j in 0..1)
    # bias[p, e*10 + 2 + i] = b2[e, 128i + p]   (i in 0..7)
    bias = nc.dram_tensor("bias", [128, EPC * (KC2 + KC1)], F32,
                          kind="ExternalInput")
    # packed out: per tile block of KC1*tn columns, [p, i*tn + c]
    outP = nc.dram_tensor("outP", [128, xcols], F32, kind="ExternalOutput")

    with tile.TileContext(nc) as tc:
        with (
            tc.tile_pool(name="const", bufs=1) as cpool,
            tc.tile_pool(name="w1p", bufs=2) as w1pool,
            tc.tile_pool(name="w2p", bufs=2) as w2pool,
            tc.tile_pool(name="xp", bufs=3) as xpool,
            tc.tile_pool(name="zp", bufs=3) as zpool,
            tc.tile_pool(name="hp", bufs=2) as hpool,
            tc.tile_pool(name="op", bufs=2) as opool,
            tc.tile_pool(name="ph", bufs=2, space="PSUM") as phpool,
            tc.tile_pool(name="po", bufs=4, space="PSUM") as popool,
        ):
            bias_t = None
            w1t = w2t = None
            cur_s = -1
            xoff = 0
            for q, (s, t0, tn) in enumerate(seq):
                new_s = s != cur_s
                cur_s = s
                if new_s:
                    w1t = w1pool.tile([128, KC1, H], MM_DT, tag="w1t")
                    nc.sync.dma_start(
                        w1t[:], w1[:, s].rearrange("p (k m) -> p k m", k=KC1))

                xt = xpool.tile([128, KC1, tn], MM_DT, tag="xt")
                nc.sync.dma_start(
                    xt[:],
                    xg[:, xoff:xoff + KC1 * tn].rearrange(
                        "p (k c) -> p k c", k=KC1))
                c0 = s * cap + t0
                zt = zpool.tile([128, tn], F32, tag="zt")
                nc.sync.dma_start(zt[:], zg[:, c0:c0 + tn])
                if bias_t is None:
                    bias_t = cpool.tile([128, EPC * (KC2 + KC1)], F32)
                    nc.sync.dma_start(bias_t[:], bias[:])
                if new_s:
                    w2t = w2pool.tile([128, KC2, D], MM_DT, tag="w2t")
                    nc.sync.dma_start(
                        w2t[:], w2[:, s].rearrange("p (j m) -> p j m", j=KC2))

                ht = hpool.tile([128, KC2, tn], MM_DT, tag="ht")
                for j in range(KC2):
                    ph = phpool.tile([128, tn], F32, tag="ph")
                    for k in range(KC1):
                        nc.tensor.matmul(
                            ph[:], w1t[:, k, 128 * j:128 * (j + 1)],
                            xt[:, k, :],
                            start=(k == 0), stop=(k == KC1 - 1))
                    nc.scalar.activation(
                        ht[:, j, :], ph[:],
                        mybir.ActivationFunctionType.Relu,
                        bias=bias_t[:, s * 10 + j:s * 10 + j + 1])

                ot = opool.tile([128, KC1, tn], F32, tag="ot")
                for i in range(KC1):
                    po = popool.tile([128, tn], F32, tag="po")
                    for j in range(KC2):
                        nc.tensor.matmul(
                            po[:], w2t[:, j, 128 * i:128 * (i + 1)],
                            ht[:, j, :],
                            start=(j == 0), stop=(j == KC2 - 1))
                    # (o + b2) * z
                    nc.vector.scalar_tensor_tensor(
                        ot[:, i, :], po[:],
                        bias_t[:, s * 10 + 2 + i:s * 10 + 3 + i],
                        zt[:],
                        mybir.AluOpType.add, mybir.AluOpType.mult)
                    # + x (residual)
                    nc.vector.tensor_add(ot[:, i, :], ot[:, i, :],
                                         xt[:, i, :])
                nc.sync.dma_start(
                    outP[:, xoff:xoff + KC1 * tn].rearrange(
                        "p (k c) -> p k c", k=KC1),
                    ot[:])

                xoff += KC1 * tn

    nc.compile()
    _build_cache[key] = nc
    return nc


def kernel(x, y_idx, y, z, W1, b1, W2, b2):
    x = np.ascontiguousarray(np.asarray(x, dtype=np.float32))
    z = np.asarray(z, dtype=np.float32)
    W1 = np.asarray(W1, dtype=np.float32)
    b1 = np.asarray(b1, dtype=np.float32)
    W2 = np.asarray(W2, dtype=np.float32)
    b2 = np.asarray(b2, dtype=np.float32)
    e = np.asarray(y_idx).reshape(-1).astype(np.int64)
    B = x.shape[0]

    idxs = [np.flatnonzero(e == k) for k in range(NB)]
    cap = max(128, -(-max(len(i) for i in idxs) // 128) * 128)
    C = EPC * cap
    seq = _tile_seq(cap)
    xcols = KC1 * cap * EPC

    nc = _build(cap)

    nbias = EPC * (KC2 + KC1)
    in_maps = []
    for c in range(NCORES):
        xg = np.zeros((128, xcols), np.float32)
        zg = np.zeros((128, C), np.float32)
        bias = np.zeros((128, nbias), np.float32)
        w1 = np.empty((128, EPC, KC1 * H), np.float32)
        w2 = np.empty((128, EPC, KC2 * D), np.float32)
        for s in range(EPC):
            k = EPC * c + s
            idx = idxs[k]
            n = len(idx)
            zg[:, s * cap:s * cap + n] = z[idx, k][None, :]
            bias[:, s * 10:s * 10 + KC2] = b1[k].reshape(KC2, 128).T
            bias[:, s * 10 + KC2:s * 10 + KC2 + KC1] = b2[k].reshape(KC1, 128).T
            w1[:, s] = W1[k].reshape(KC1, 128, H).transpose(1, 0, 2).reshape(
                128, KC1 * H)
            w2[:, s] = W2[k].reshape(KC2, 128, D).transpose(1, 0, 2).reshape(
                128, KC2 * D)
        xoff = 0
        for s, t0, tn in seq:
            k = EPC * c + s
            seg = idxs[k][t0:t0 + tn]
            n = len(seg)
            if n:
                full = np.zeros((128, KC1, tn), np.float32)
                full[:, :, :n] = x[seg].reshape(n, KC1, 128).transpose(2, 1, 0)
                xg[:, xoff:xoff + KC1 * tn] = full.reshape(128, KC1 * tn)
            xoff += KC1 * tn
        in_maps.append({"xg": xg, "zg": zg, "w1": w1, "w2": w2, "bias": bias})

    res = run_bass_kernel_spmd(nc, in_maps, core_ids=list(range(NCORES)))
    global LAST_RESULTS
    LAST_RESULTS = res

    out = np.empty((B, D), np.float32)
    for c in range(NCORES):
        outP = res.results[c]["outP"]
        xoff = 0
        for s, t0, tn in seq:
            k = EPC * c + s
            seg = idxs[k][t0:t0 + tn]
            n = len(seg)
            if n:
                blk = outP[:, xoff:xoff + KC1 * tn].reshape(128, KC1, tn)
                # blk[p, i, c] = out[token c, 128i + p]
                out[seg] = blk[:, :, :n].transpose(2, 1, 0).reshape(n, D)
            xoff += KC1 * tn
    return out


# revision 8
# speedup vs baseline: 1.6100x; 1.4629x over previous
"""Trainium2 Bass kernel for nn_DecSwitchedFC (MoE hard routing).

Math (per token b, expert e = y_idx[b]):
    out[b] = x[b] + z[b, e] * (relu(x[b] @ W1[e] + b1[e]) @ W2[e] + b2[e])

Strategy: expert-parallel over 8 NeuronCores, 2 experts per core.  The host
routes tokens to their experts (pure data movement — gather/scatter and
transpose), each core runs the two dense bottleneck FCs for its experts'
tokens on the tensor engine, applies bias/relu/route-scale/residual on the
scalar/vector engines, and the host scatters rows back.  Only the selected
expert is computed per token (1/16 of the reference FLOPs).

Device data layout is fully "feature-major" (d or h on partitions, tokens on
the free axis) so no on-device transposes are needed:
    h^T[256, n]  = W1[e]^T(lhsT=W1) @ x^T          (K=1024, 8 chunks)
    o^T[1024, n] = W2[e]^T(lhsT=W2) @ relu(h^T+b1) (K=256, 2 chunks)
    out^T        = (o^T + b2) * z + x^T

All host-side arrays are packed so every DMA is a single issue with 128
descriptors of contiguous bytes (one per SBUF partition) — DMA issue rate,
not bandwidth, limits the pipeline ramp otherwise.
"""

import numpy as np

import concourse.bacc as bacc
import concourse.mybir as mybir
import concourse.tile as tile
from concourse.bass_utils import run_bass_kernel_spmd

D = 1024        # model dim
H = 256         # bottleneck dim
NB = 16         # n experts
NCORES = 8
EPC = NB // NCORES   # experts per core
TILE_N = 512    # token-tile width (fp32 moving-operand max)
KC1 = D // 128  # contraction chunks for x @ W1
KC2 = H // 128  # contraction chunks for h @ W2
F32 = mybir.dt.float32
# Matmul operand dtype: float32 (exact, 4 cyc/row) or float32r
# (TF32-like ~1e-4 relative error, 1 cyc/row).
MM_DT = mybir.dt.float32r

_build_cache: dict[tuple, object] = {}
LAST_RESULTS = None  # BassKernelResults of the most recent run (for profiling)


def _tile_seq(cap: int):
    """Global tile order: small tile first (fast PE start) and small tile
    last (short drain tail).  Returns [(s, t0, tn), ...]."""
    tiles = [(t0, min(TILE_N, cap - t0)) for t0 in range(0, cap, TILE_N)]
    first = sorted(tiles, key=lambda t: t[1])          # ascending width
    last = sorted(tiles, key=lambda t: -t[1])          # descending width
    seq = [(0, t0, tn) for t0, tn in first]
    seq += [(1, t0, tn) for t0, tn in last]
    return seq


def _build(cap: int):
    key = (cap, MM_DT)
    if key in _build_cache:
        return _build_cache[key]
    C = EPC * cap
    seq = _tile_seq(cap)
    xcols = KC1 * cap * EPC   # total packed x / out columns

    nc = bacc.Bacc("TRN2", target_bir_lowering=False, debug=False)

    xg = nc.dram_tensor("xg", [128, xcols], MM_DT, kind="ExternalInput")
    zg = nc.dram_tensor("zg", [128, C], F32, kind="ExternalInput")
    w1 = nc.dram_tensor("w1", [128, EPC, KC1 * H], MM_DT, kind="ExternalInput")
    w2 = nc.dram_tensor("w2", [128, EPC, KC2 * D], MM_DT, kind="ExternalInput")
    # bias[p, e*10 + j]     = b1[e, 128j + p]   (j in 0..1)
    # bias[p, e*10 + 2 + i] = b2[e, 128i + p]   (i in 0..7)
    bias = nc.dram_tensor("bias", [128, EPC * (KC2 + KC1)], F32,
                          kind="ExternalInput")
    # packed out: per tile block of KC1*tn columns, [p, i*tn + c]
    outP = nc.dram_tensor("outP", [128, xcols], F32, kind="ExternalOutput")

    with tile.TileContext(nc) as tc:
        with (
            tc.tile_pool(name="const", bufs=1) as cpool,
            tc.tile_pool(name="w1p", bufs=2) as w1pool,
            tc.tile_pool(name="w2p", bufs=2) as w2pool,
            tc.tile_pool(name="xp", bufs=3) as xpool,
            tc.tile_pool(name="zp", bufs=3) as zpool,
            tc.tile_pool(name="hp", bufs=2) as hpool,
            tc.tile_pool(name="op", bufs=2) as opool,
            tc.tile_pool(name="ph", bufs=2, space="PSUM") as phpool,
            tc.tile_pool(name="po", bufs=4, space="PSUM") as popool,
        ):
            bias_t = None
            w1t = w2t = None
            cur_s = -1
            xoff = 0
            for q, (s, t0, tn) in enumerate(seq):
                new_s = s != cur_s
                cur_s = s
                if new_s:
                    w1t = w1pool.tile([128, KC1, H], MM_DT, tag="w1t")
                    nc.sync.dma_start(
                        w1t[:], w1[:, s].rearrange("p (k m) -> p k m", k=KC1))

                xt = xpool.tile([128, KC1, tn], MM_DT, tag="xt")
                nc.sync.dma_start(
                    xt[:],
                    xg[:, xoff:xoff + KC1 * tn].rearrange(
                        "p (k c) -> p k c", k=KC1))
                c0 = s * cap + t0
                zt = zpool.tile([128, tn], F32, tag="zt")
                nc.sync.dma_start(zt[:], zg[:, c0:c0 + tn])
                if bias_t is None:
                    bias_t = cpool.tile([128, EPC * (KC2 + KC1)], F32)
                    nc.sync.dma_start(bias_t[:], bias[:])
                if new_s:
                    w2t = w2pool.tile([128, KC2, D], MM_DT, tag="w2t")
                    nc.sync.dma_start(
                        w2t[:], w2[:, s].rearrange("p (j m) -> p j m", j=KC2))

                ht = hpool.tile([128, KC2, tn], MM_DT, tag="ht")
                for j in range(KC2):
                    ph = phpool.tile([128, tn], F32, tag="ph")
                    for k in range(KC1):
                        nc.tensor.matmul(
                            ph[:], w1t[:, k, 128 * j:128 * (j + 1)],
                            xt[:, k, :],
                            start=(k == 0), stop=(k == KC1 - 1))
                    nc.scalar.activation(
                        ht[:, j, :], ph[:],
                        mybir.ActivationFunctionType.Relu,
                        bias=bias_t[:, s * 10 + j:s * 10 + j + 1])

                ot = opool.tile([128, KC1, tn], F32, tag="ot")
                for i in range(KC1):
                    po = popool.tile([128, tn], F32, tag="po")
                    for j in range(KC2):
                        nc.tensor.matmul(
                            po[:], w2t[:, j, 128 * i:128 * (i + 1)],
                            ht[:, j, :],
                            start=(j == 0), stop=(j == KC2 - 1))
                    # (o + b2) * z
                    nc.vector.scalar_tensor_tensor(
                        ot[:, i, :], po[:],
                        bias_t[:, s * 10 + 2 + i:s * 10 + 3 + i],
                        zt[:],
                        mybir.AluOpType.add, mybir.AluOpType.mult)
                    # + x (residual)
                    nc.vector.tensor_add(ot[:, i, :], ot[:, i, :],
                                         xt[:, i, :])
                nc.sync.dma_start(
                    outP[:, xoff:xoff + KC1 * tn].rearrange(
                        "p (k c) -> p k c", k=KC1),
                    ot[:])

                xoff += KC1 * tn

    nc.compile()
    _build_cache[key] = nc
    return nc


def kernel(x, y_idx, y, z, W1, b1, W2, b2):
    x = np.ascontiguousarray(np.asarray(x, dtype=np.float32))
    z = np.asarray(z, dtype=np.float32)
    W1 = np.asarray(W1, dtype=np.float32)
    b1 = np.asarray(b1, dtype=np.float32)
    W2 = np.asarray(W2, dtype=np.float32)
    b2 = np.asarray(b2, dtype=np.float32)
    e = np.asarray(y_idx).reshape(-1).astype(np.int64)
    B = x.shape[0]

    idxs = [np.flatnonzero(e == k) for k in range(NB)]
    cap = max(128, -(-max(len(i) for i in idxs) // 128) * 128)
    C = EPC * cap
    seq = _tile_seq(cap)
    xcols = KC1 * cap * EPC

    nc = _build(cap)

    nbias = EPC * (KC2 + KC1)
    in_maps = []
    for c in range(NCORES):
        xg = np.zeros((128, xcols), np.float32)
        zg = np.zeros((128, C), np.float32)
        bias = np.zeros((128, nbias), np.float32)
        w1 = np.empty((128, EPC, KC1 * H), np.float32)
        w2 = np.empty((128, EPC, KC2 * D), np.float32)
        for s in range(EPC):
            k = EPC * c + s
            idx = idxs[k]
            n = len(idx)
            zg[:, s * cap:s * cap + n] = z[idx, k][None, :]
            bias[:, s * 10:s * 10 + KC2] = b1[k].reshape(KC2, 128).T
            bias[:, s * 10 + KC2:s * 10 + KC2 + KC1] = b2[k].reshape(KC1, 128).T
            w1[:, s] = W1[k].reshape(KC1, 128, H).transpose(1, 0, 2).reshape(
                128, KC1 * H)
            w2[:, s] = W2[k].reshape(KC2, 128, D).transpose(1, 0, 2).reshape(
                128, KC2 * D)
        xoff = 0
        for s, t0, tn in seq:
            k = EPC * c + s
            seg = idxs[k][t0:t0 + tn]
            n = len(seg)
            if n:
                full = np.zeros((128, KC1, tn), np.float32)
                full[:, :, :n] = x[seg].reshape(n, KC1, 128).transpose(2, 1, 0)
                xg[:, xoff:xoff + KC1 * tn] = full.reshape(128, KC1 * tn)
            xoff += KC1 * tn
        in_maps.append({"xg": xg, "zg": zg, "w1": w1, "w2": w2, "bias": bias})

    res = run_bass_kernel_spmd(nc, in_maps, core_ids=list(range(NCORES)))
    global LAST_RESULTS
    LAST_RESULTS = res

    out = np.empty((B, D), np.float32)
    for c in range(NCORES):
        outP = res.results[c]["outP"]
        xoff = 0
        for s, t0, tn in seq:
            k = EPC * c + s
            seg = idxs[k][t0:t0 + tn]
            n = len(seg)
            if n:
                blk = outP[:, xoff:xoff + KC1 * tn].reshape(128, KC1, tn)
                # blk[p, i, c] = out[token c, 128i + p]
                out[seg] = blk[:, :, :n].transpose(2, 1, 0).reshape(n, D)
            xoff += KC1 * tn
    return out
